# revision 19
# baseline (speedup 1.0000x reference)
"""Distributed Bass kernel for nn_AttentionLayer (B=2, S=2048, HID=1024, NH=16).

Sharding: core c of 8 handles batch b = c//4 and heads [4*(c%4), 4*(c%4)+4).
 - QKV projections + causal attention computed fully locally per (batch, head-group).
 - attn probabilities ([B, NH, S, S]) written directly to per-core output shards.
 - ctx^T shards exchanged with an AllToAll over 4-core batch groups, after which
   each core computes a disjoint 512-row slice of the output projection.

Host-side work is layout only: inputs are sliced/transposed per core, outputs
are concatenated. All FLOPs run on the NeuronCores.
"""

import math
import os
from contextlib import ExitStack

import numpy as np

import concourse.bass as bass
import concourse.tile as tile
from concourse import bacc, bass_utils, mybir
from concourse.masks import make_identity

# ---- problem constants (hardcoded per harness) ----
B, S, HID, NH, HD = 2, 2048, 1024, 16, 64
N_CORES, CPB = 8, 4  # total cores, cores per batch
HG = NH // CPB  # heads per core = 4
DG = HG * HD  # ctx dims per core = 256
P = 128
F32 = mybir.dt.float32
F32R = mybir.dt.float32r
NEG_BIG = -1.0e30
SCALE = float(HD) ** 0.5  # reference: scores = qk / (HD ** -0.5) == qk * 8
RG = [[0, 1, 2, 3], [4, 5, 6, 7]]
AFT = mybir.ActivationFunctionType

# runtime-config (test.py may override via env)
USE_FP32R_SCORES = os.environ.get("K_FP32R_SCORES", "1") == "1"
USE_FP32R_OTHER = os.environ.get("K_FP32R_OTHER", "1") == "1"

LAST_RESULT = None  # test.py reads exec_time_ns from here


def _build(s=S, fp32r_scores=USE_FP32R_SCORES, fp32r_other=USE_FP32R_OTHER,
           add_bias_v=False, add_bias_o=False):
    ST = s // P          # s-tiles
    SC = s // 512        # 512-wide chunks
    IC = HID // P        # hid chunks = 8
    NQG = ST // 4        # q groups (4 tiles each)
    RS = s // CPB        # out rows per core
    RT = RS // P

    DTS = F32R if fp32r_scores else F32   # scores matmul operand dtype
    DTO = F32R if fp32r_other else F32    # other matmul operand dtype

    nc = bacc.Bacc("TRN2", debug=False, num_devices=N_CORES)

    xT = nc.dram_tensor("xT", [HID, s], F32, kind="ExternalInput")
    wqT = nc.dram_tensor("wqT", [HID, DG], F32, kind="ExternalInput")
    wkT = nc.dram_tensor("wkT", [HID, DG], F32, kind="ExternalInput")
    wvT = nc.dram_tensor("wvT", [HID, DG], F32, kind="ExternalInput")
    OC = HID // CPB  # out-projection columns per core = 256
    woT = nc.dram_tensor("woT", [HID, OC], F32, kind="ExternalInput")
    bq = nc.dram_tensor("bq", [DG], F32, kind="ExternalInput")
    bk = nc.dram_tensor("bk", [DG], F32, kind="ExternalInput")
    bv = nc.dram_tensor("bv", [DG], F32, kind="ExternalInput")
    bo = nc.dram_tensor("bo", [OC], F32, kind="ExternalInput")
    attn_o = nc.dram_tensor("attn_o", [HG, s, s], F32, kind="ExternalOutput")
    out_o = nc.dram_tensor("out_o", [s, OC], F32, kind="ExternalOutput")
    # per-q-group collective bounce buffers (contiguous per chunk)
    cc_in = [nc.dram_tensor(f"cc_in{g}", [DG, 4 * P], F32) for g in range(s // P // 4)]
    cc_out = [
        nc.dram_tensor(f"cc_out{g}", [CPB * DG, 4 * P], F32) for g in range(s // P // 4)
    ]

    ao = attn_o.ap()
    oo = out_o.ap()

    with tile.TileContext(nc) as tc:
        with ExitStack() as ctx:
            consts = ctx.enter_context(tc.tile_pool(name="consts", bufs=1))
            acts = ctx.enter_context(tc.tile_pool(name="acts", bufs=1))
            ps_mm = ctx.enter_context(tc.tile_pool(name="ps_mm", bufs=4, space="PSUM"))
            ps_tp = ctx.enter_context(tc.tile_pool(name="ps_tp", bufs=2, space="PSUM"))
            ps_ctx = ctx.enter_context(tc.tile_pool(name="ps_ctx", bufs=2, space="PSUM"))

            # ---- constants ----
            identity = consts.tile([P, P], F32)
            make_identity(nc, identity[:])
            cmask = consts.tile([P, P], F32)
            nc.gpsimd.memset(cmask[:], 0.0)
            # cmask[x, y] = (x - y >= 0) ? 0 : NEG_BIG   (mask strictly-upper)
            nc.gpsimd.affine_select(
                out=cmask[:], in_=cmask[:],
                compare_op=mybir.AluOpType.is_ge, fill=NEG_BIG,
                base=0, pattern=[[-1, P]], channel_multiplier=1,
            )
            bq_sb = consts.tile([P, 2], F32)
            nc.sync.dma_start(bq_sb[:], bq.ap().rearrange("(o p) -> p o", p=P))
            bk_sb = consts.tile([P, 2], F32)
            nc.sync.dma_start(bk_sb[:], bk.ap().rearrange("(o p) -> p o", p=P))
            bv_bc = bo_bc = None
            if add_bias_v:
                bv_bc = consts.tile([P, DG], F32)
                nc.sync.dma_start(bv_bc[0:1, :], bv.ap()[None, :])
                nc.gpsimd.partition_broadcast(bv_bc[:], bv_bc[0:1, :])
            if add_bias_o:
                bo_bc = consts.tile([P, OC], F32)
                nc.sync.dma_start(bo_bc[0:1, :], bo.ap()[None, :])
                nc.gpsimd.partition_broadcast(bo_bc[:], bo_bc[0:1, :])

            # ---- persistent activations ----
            qT_sb = acts.tile([P, 2, s], DTS)  # q^T  [head-dim part, s]
            kT_sb = acts.tile([P, 2, s], DTS)
            v_sb = acts.tile([P, ST, DG], DTO)  # v    [s part, head dims]
            ctxT_sb = acts.tile([P, 2, s], F32)  # ctx^T

            # ---- phase 1: load xT + weights, projections ----
            with tc.tile_pool(name="wts", bufs=1) as wts:
                xT_sb = wts.tile([P, IC, s], DTO)
                nc.gpsimd.dma_start(xT_sb[:], xT.ap().rearrange("(c p) t -> p c t", p=P))
                wqT_sb = wts.tile([P, IC, DG], DTO)
                nc.gpsimd.dma_start(wqT_sb[:], wqT.ap().rearrange("(c p) o -> p c o", p=P))
                wkT_sb = wts.tile([P, IC, DG], DTO)
                nc.gpsimd.dma_start(wkT_sb[:], wkT.ap().rearrange("(c p) o -> p c o", p=P))
                wvT_sb = wts.tile([P, IC, DG], DTO)
                nc.gpsimd.dma_start(wvT_sb[:], wvT.ap().rearrange("(c p) o -> p c o", p=P))

                # q^T, k^T: [256, s] = W_slice @ x^T, via lhsT = W^T chunks
                for wT_sb, dst, b_sb in ((wqT_sb, qT_sb, bq_sb), (wkT_sb, kT_sb, bk_sb)):
                    for ob in range(2):
                        for sc in range(SC):
                            ps = ps_mm.tile([P, 512], F32, tag="mm", name="ps_proj")
                            for ic in range(IC):
                                nc.tensor.matmul(
                                    ps[:],
                                    wT_sb[:, ic, ob * P:(ob + 1) * P],
                                    xT_sb[:, ic, sc * 512:(sc + 1) * 512],
                                    start=(ic == 0), stop=(ic == IC - 1),
                                )
                            nc.scalar.activation(
                                dst[:, ob, sc * 512:(sc + 1) * 512], ps[:],
                                AFT.Identity, bias=b_sb[:, ob:ob + 1], scale=1.0,
                            )

                # v: [s, 256] = x @ Wv_slice^T, via lhsT = x^T chunks
                for s_t in range(ST):
                    ps = ps_mm.tile([P, 512], F32, tag="mm", name="ps_v")
                    for ic in range(IC):
                        nc.tensor.matmul(
                            ps[:, :DG],
                            xT_sb[:, ic, s_t * P:(s_t + 1) * P],
                            wvT_sb[:, ic, :],
                            start=(ic == 0), stop=(ic == IC - 1),
                        )
                    if add_bias_v:
                        nc.vector.tensor_add(v_sb[:, s_t, :], ps[:, :DG], bv_bc[:])
                    else:
                        nc.any.tensor_copy(v_sb[:, s_t, :], ps[:, :DG])

            # phase-2/3 pools opened after wts closed so its space is reclaimed
            attn_pool = ctx.enter_context(tc.tile_pool(name="attn", bufs=2))
            stage = ctx.enter_context(tc.tile_pool(name="stage", bufs=3))
            stats = ctx.enter_context(tc.tile_pool(name="stats", bufs=4))
            outp = ctx.enter_context(tc.tile_pool(name="outp", bufs=3))

            # woT used only in phase 3
            woT_sb = outp.tile([P, IC, OC], DTO, tag="woT", bufs=1)
            nc.gpsimd.dma_start(woT_sb[:], woT.ap().rearrange("(c p) o -> p c o", p=P))

            # ---- phase 2 + 3 interleaved: q-group outer so the per-group
            # AllGather + out-projection overlap later groups' attention ----
            for g in range(NQG):
                for h in range(HG):
                    hp = 64 * (h % 2)  # partition offset of head in qT/kT blocks
                    hb = h // 2        # o-block of this head
                    attn_t = attn_pool.tile([P, 4, s], F32, tag="attn", name="attn_t")
                    for tq in range(4 * g, 4 * g + 4):
                        L = (tq + 1) * P
                        nch = (L + 511) // 512
                        row = attn_t[:, tq - 4 * g, :]
                        # online (per-chunk) softmax: each chunk exps against
                        # its own max so PSUM frees immediately; a per-chunk
                        # rescale factor fixes everything up at normalize time.
                        cmax = stats.tile([P, 4], F32, tag="cmax", name="cmax")
                        nm8c = stats.tile([P, 4], F32, tag="nm8c", name="nm8c")
                        zp = stats.tile([P, 4], F32, tag="zp", name="zp")
                        e8 = stats.tile([P, 4], F32, tag="e8", name="e8")
                        zw = stats.tile([P, 4], F32, tag="zw", name="zw")
                        fac = stats.tile([P, 4], F32, tag="fac", name="fac")
                        negmax = stats.tile([P, 1], F32, tag="negmax", name="negmax")
                        negm8 = stats.tile([P, 1], F32, tag="negm8", name="negm8")
                        z = stats.tile([P, 1], F32, tag="z", name="z")
                        invz = stats.tile([P, 1], F32, tag="invz", name="invz")
                        for kc in range(nch):
                            w = min(512, L - kc * 512)
                            ps = ps_mm.tile([P, 512], F32, tag="mm", name="ps_s")
                            nc.tensor.matmul(
                                ps[:, :w],
                                qT_sb[hp:hp + 64, hb, tq * P:(tq + 1) * P],
                                kT_sb[hp:hp + 64, hb, kc * 512:kc * 512 + w],
                                start=True, stop=True,
                            )
                            if kc == nch - 1:
                                nc.vector.tensor_add(
                                    ps[:, w - P:w], ps[:, w - P:w], cmask[:]
                                )
                            nc.vector.reduce_max(
                                cmax[:, kc:kc + 1], ps[:, :w],
                                axis=mybir.AxisListType.X,
                            )
                            nc.vector.tensor_scalar_mul(
                                nm8c[:, kc:kc + 1], cmax[:, kc:kc + 1], -SCALE
                            )
                            nc.scalar.activation(
                                row[:, kc * 512:kc * 512 + w], ps[:, :w],
                                AFT.Exp, bias=nm8c[:, kc:kc + 1], scale=SCALE,
                                accum_out=zp[:, kc:kc + 1],
                            )
                        # combine: M = max_kc cmax; e8 = exp(SCALE*(cmax-M));
                        # Z = sum e8*zp; row[kc] *= e8[kc]/Z
                        nc.vector.reduce_max(
                            negmax[:], cmax[:, :nch],
                            axis=mybir.AxisListType.X, negate=True,
                        )
                        nc.vector.tensor_scalar_mul(negm8[:], negmax[:], SCALE)
                        nc.scalar.activation(
                            e8[:, :nch], cmax[:, :nch],
                            AFT.Exp, bias=negm8[:, 0:1], scale=SCALE,
                        )
                        nc.vector.tensor_mul(zw[:, :nch], zp[:, :nch], e8[:, :nch])
                        nc.vector.reduce_sum(
                            z[:], zw[:, :nch], axis=mybir.AxisListType.X
                        )
                        nc.vector.reciprocal(invz[:], z[:])
                        nc.vector.tensor_scalar_mul(
                            fac[:, :nch], e8[:, :nch], invz[:, 0:1]
                        )
                        for kc in range(nch):
                            w = min(512, L - kc * 512)
                            nc.gpsimd.tensor_scalar_mul(
                                row[:, kc * 512:kc * 512 + w],
                                row[:, kc * 512:kc * 512 + w],
                                fac[:, kc:kc + 1],
                            )
                        nc.sync.dma_start(
                            ao[h, tq * P:(tq + 1) * P, :L], row[:, :L]
                        )

                    # AV: ctx^T[64, 512] for this q-group, contract over k-tiles
                    ctx_ps = ps_ctx.tile([64, 512], F32, tag="ctx", name="ctx_ps")
                    for j in range(4 * g + 4):
                        t0 = max(0, j - 4 * g)  # first valid local q-tile
                        stg = stage.tile([P, 512], DTO, tag="stg", name="stg")
                        tp = ps_tp.tile([P, 512], F32, tag="tp", name="tp")
                        for tl in range(t0, 4):
                            nc.tensor.transpose(
                                tp[:, tl * P:(tl + 1) * P],
                                attn_t[:, tl, j * P:(j + 1) * P],
                                identity[:],
                            )
                        if t0 > 0:
                            nc.gpsimd.memset(
                                stg[:, :t0 * P].bitcast(mybir.dt.uint32), 0
                            )
                            nc.any.tensor_copy(stg[:, t0 * P:], tp[:, t0 * P:])
                        else:
                            nc.any.tensor_copy(stg[:], tp[:])
                        nc.tensor.matmul(
                            ctx_ps[:],
                            v_sb[:, j, h * 64:(h + 1) * 64],
                            stg[:],
                            start=(j == 0), stop=(j == 4 * g + 3),
                        )
                    nc.any.tensor_copy(
                        ctxT_sb[hp:hp + 64, hb, g * 512:(g + 1) * 512], ctx_ps[:]
                    )

                # ---- per-group AllGather + out-projection (columns slice) ----
                for blk in range(2):
                    nc.sync.dma_start(
                        cc_in[g].ap()[blk * P:(blk + 1) * P, :],
                        ctxT_sb[:, blk, g * 512:(g + 1) * 512],
                    )
                nc.gpsimd.collective_compute(
                    "AllGather", mybir.AluOpType.bypass, replica_groups=RG,
                    ins=[cc_in[g].ap().opt()], outs=[cc_out[g].ap().opt()],
                )
                ccv = cc_out[g].ap().rearrange("(c p) t -> p c t", p=P)
                for st_l in range(4):
                    ctx_l = outp.tile([P, IC, P], DTO, tag="ctxl", name="ctx_l")
                    nc.gpsimd.dma_start(
                        ctx_l[:], ccv[:, :, st_l * P:(st_l + 1) * P]
                    )
                    ps = ps_mm.tile([P, 512], F32, tag="mm", name="ps_o")
                    for ic in range(IC):
                        nc.tensor.matmul(
                            ps[:, :OC],
                            ctx_l[:, ic, :],
                            woT_sb[:, ic, :],
                            start=(ic == 0), stop=(ic == IC - 1),
                        )
                    ot = outp.tile([P, OC], F32, tag="ot", name="ot")
                    if add_bias_o:
                        nc.vector.tensor_add(ot[:], ps[:, :OC], bo_bc[:])
                    else:
                        nc.any.tensor_copy(ot[:], ps[:, :OC])
                    s_t = g * 4 + st_l
                    nc.sync.dma_start(oo[s_t * P:(s_t + 1) * P, :], ot[:])

    nc.compile()
    return nc


def make_in_maps(x, Wq, bq, Wk, bk, Wv, bv, Wo, bo, s=S):
    """Host-side sharding: slicing + layout transposes only."""
    x = np.asarray(x, dtype=np.float32)
    OC = HID // CPB
    in_maps = []
    xT = [np.ascontiguousarray(x[b].T) for b in range(B)]
    for c in range(N_CORES):
        b, hg = c // CPB, c % CPB
        sl = slice(hg * DG, (hg + 1) * DG)
        osl = slice(hg * OC, (hg + 1) * OC)
        in_maps.append({
            "xT": xT[b],
            "wqT": np.ascontiguousarray(np.asarray(Wq, np.float32)[sl].T),
            "wkT": np.ascontiguousarray(np.asarray(Wk, np.float32)[sl].T),
            "wvT": np.ascontiguousarray(np.asarray(Wv, np.float32)[sl].T),
            "woT": np.ascontiguousarray(np.asarray(Wo, np.float32)[osl].T),
            "bq": np.ascontiguousarray(np.asarray(bq, np.float32)[sl]),
            "bk": np.ascontiguousarray(np.asarray(bk, np.float32)[sl]),
            "bv": np.ascontiguousarray(np.asarray(bv, np.float32)[sl]),
            "bo": np.ascontiguousarray(np.asarray(bo, np.float32)[osl]),
        })
    return in_maps


def assemble(results, s=S):
    OC = HID // CPB
    out = np.empty((B, s, HID), dtype=np.float32)
    attn = np.empty((B, NH, s, s), dtype=np.float32)
    for c in range(N_CORES):
        b, hg = c // CPB, c % CPB
        attn[b, hg * HG:(hg + 1) * HG] = results[c]["attn_o"]
        out[b, :, hg * OC:(hg + 1) * OC] = results[c]["out_o"]
    return out, attn


def kernel(x, Wq, bq, Wk, bk, Wv, bv, Wo, bo):
    global LAST_RESULT
    add_bias_v = bool(np.any(np.asarray(bv)))
    add_bias_o = bool(np.any(np.asarray(bo)))
    nc = _build(add_bias_v=add_bias_v, add_bias_o=add_bias_o)
    in_maps = make_in_maps(x, Wq, bq, Wk, bk, Wv, bv, Wo, bo)
    res = bass_utils.run_bass_kernel_spmd(
        nc, in_maps, core_ids=list(range(N_CORES)),
    )
    LAST_RESULT = res
    return assemble(res.results)


# revision 20
# speedup vs baseline: 2.3188x; 2.3188x over previous
"""Distributed Bass kernel for nn_AttentionLayer (B=2, S=2048, HID=1024, NH=16).

Sharding: core c of 8 handles batch b = c//4 and heads [4*(c%4), 4*(c%4)+4).
 - QKV projections + causal attention computed fully locally per (batch, head-group).
 - attn probabilities ([B, NH, S, S]) written directly to per-core output shards.
 - ctx^T shards exchanged with an AllToAll over 4-core batch groups, after which
   each core computes a disjoint 512-row slice of the output projection.

Host-side work is layout only: inputs are sliced/transposed per core, outputs
are concatenated. All FLOPs run on the NeuronCores.
"""

import math
import os
from contextlib import ExitStack

import numpy as np

import concourse.bass as bass
import concourse.tile as tile
from concourse import bacc, bass_utils, mybir
from concourse.masks import make_identity

# ---- problem constants (hardcoded per harness) ----
B, S, HID, NH, HD = 2, 2048, 1024, 16, 64
N_CORES, CPB = 8, 4  # total cores, cores per batch
HG = NH // CPB  # heads per core = 4
DG = HG * HD  # ctx dims per core = 256
P = 128
F32 = mybir.dt.float32
F32R = mybir.dt.float32r
NEG_BIG = -1.0e30
SCALE = float(HD) ** 0.5  # reference: scores = qk / (HD ** -0.5) == qk * 8
RG = [[0, 1, 2, 3], [4, 5, 6, 7]]
AFT = mybir.ActivationFunctionType

# runtime-config (test.py may override via env)
USE_FP32R_SCORES = os.environ.get("K_FP32R_SCORES", "1") == "1"
USE_FP32R_OTHER = os.environ.get("K_FP32R_OTHER", "1") == "1"

LAST_RESULT = None  # test.py reads exec_time_ns from here


def _build(s=S, fp32r_scores=USE_FP32R_SCORES, fp32r_other=USE_FP32R_OTHER,
           add_bias_v=False, add_bias_o=False):
    ST = s // P          # s-tiles
    SC = s // 512        # 512-wide chunks
    IC = HID // P        # hid chunks = 8
    NQG = ST // 4        # q groups (4 tiles each)
    RS = s // CPB        # out rows per core
    RT = RS // P

    DTS = F32R if fp32r_scores else F32   # scores matmul operand dtype
    DTO = F32R if fp32r_other else F32    # other matmul operand dtype

    nc = bacc.Bacc("TRN2", debug=False, num_devices=N_CORES)

    xT = nc.dram_tensor("xT", [HID, s], F32, kind="ExternalInput")
    wqT = nc.dram_tensor("wqT", [HID, DG], F32, kind="ExternalInput")
    wkT = nc.dram_tensor("wkT", [HID, DG], F32, kind="ExternalInput")
    wvT = nc.dram_tensor("wvT", [HID, DG], F32, kind="ExternalInput")
    OC = HID // CPB  # out-projection columns per core = 256
    woT = nc.dram_tensor("woT", [HID, OC], F32, kind="ExternalInput")
    bq = nc.dram_tensor("bq", [DG], F32, kind="ExternalInput")
    bk = nc.dram_tensor("bk", [DG], F32, kind="ExternalInput")
    bv = nc.dram_tensor("bv", [DG], F32, kind="ExternalInput")
    bo = nc.dram_tensor("bo", [OC], F32, kind="ExternalInput")
    attn_o = nc.dram_tensor("attn_o", [HG, s, s], F32, kind="ExternalOutput")
    out_o = nc.dram_tensor("out_o", [s, OC], F32, kind="ExternalOutput")
    # per-q-group collective bounce buffers (contiguous per chunk)
    cc_in = [nc.dram_tensor(f"cc_in{g}", [DG, 4 * P], F32) for g in range(s // P // 4)]
    cc_out = [
        nc.dram_tensor(f"cc_out{g}", [CPB * DG, 4 * P], F32) for g in range(s // P // 4)
    ]

    ao = attn_o.ap()
    oo = out_o.ap()

    with tile.TileContext(nc) as tc:
        with ExitStack() as ctx:
            consts = ctx.enter_context(tc.tile_pool(name="consts", bufs=1))
            acts = ctx.enter_context(tc.tile_pool(name="acts", bufs=1))
            ps_mm = ctx.enter_context(tc.tile_pool(name="ps_mm", bufs=4, space="PSUM"))
            ps_tp = ctx.enter_context(tc.tile_pool(name="ps_tp", bufs=2, space="PSUM"))
            ps_ctx = ctx.enter_context(tc.tile_pool(name="ps_ctx", bufs=2, space="PSUM"))

            # ---- constants ----
            identity = consts.tile([P, P], F32)
            make_identity(nc, identity[:])
            cmask = consts.tile([P, P], F32)
            nc.gpsimd.memset(cmask[:], 0.0)
            # cmask[x, y] = (x - y >= 0) ? 0 : NEG_BIG   (mask strictly-upper)
            nc.gpsimd.affine_select(
                out=cmask[:], in_=cmask[:],
                compare_op=mybir.AluOpType.is_ge, fill=NEG_BIG,
                base=0, pattern=[[-1, P]], channel_multiplier=1,
            )
            bq_sb = consts.tile([P, 2], F32)
            nc.sync.dma_start(bq_sb[:], bq.ap().rearrange("(o p) -> p o", p=P))
            bk_sb = consts.tile([P, 2], F32)
            nc.sync.dma_start(bk_sb[:], bk.ap().rearrange("(o p) -> p o", p=P))
            bv_bc = bo_bc = None
            if add_bias_v:
                bv_bc = consts.tile([P, DG], F32)
                nc.sync.dma_start(bv_bc[0:1, :], bv.ap()[None, :])
                nc.gpsimd.partition_broadcast(bv_bc[:], bv_bc[0:1, :])
            if add_bias_o:
                bo_bc = consts.tile([P, OC], F32)
                nc.sync.dma_start(bo_bc[0:1, :], bo.ap()[None, :])
                nc.gpsimd.partition_broadcast(bo_bc[:], bo_bc[0:1, :])

            # ---- persistent activations ----
            qT_sb = acts.tile([P, 2, s], DTS)  # q^T  [head-dim part, s]
            kT_sb = acts.tile([P, 2, s], DTS)
            v_sb = acts.tile([P, ST, DG], DTO)  # v    [s part, head dims]
            ctxT_sb = acts.tile([P, 2, s], F32)  # ctx^T

            # ---- phase 1: load xT + weights, projections ----
            with tc.tile_pool(name="wts", bufs=1) as wts:
                xT_sb = wts.tile([P, IC, s], DTO)
                nc.gpsimd.dma_start(xT_sb[:], xT.ap().rearrange("(c p) t -> p c t", p=P))
                wqT_sb = wts.tile([P, IC, DG], DTO)
                nc.gpsimd.dma_start(wqT_sb[:], wqT.ap().rearrange("(c p) o -> p c o", p=P))
                wkT_sb = wts.tile([P, IC, DG], DTO)
                nc.gpsimd.dma_start(wkT_sb[:], wkT.ap().rearrange("(c p) o -> p c o", p=P))
                wvT_sb = wts.tile([P, IC, DG], DTO)
                nc.gpsimd.dma_start(wvT_sb[:], wvT.ap().rearrange("(c p) o -> p c o", p=P))

                # q^T, k^T: [256, s] = W_slice @ x^T, via lhsT = W^T chunks
                for wT_sb, dst, b_sb in ((wqT_sb, qT_sb, bq_sb), (wkT_sb, kT_sb, bk_sb)):
                    for ob in range(2):
                        for sc in range(SC):
                            ps = ps_mm.tile([P, 512], F32, tag="mm", name="ps_proj")
                            for ic in range(IC):
                                nc.tensor.matmul(
                                    ps[:],
                                    wT_sb[:, ic, ob * P:(ob + 1) * P],
                                    xT_sb[:, ic, sc * 512:(sc + 1) * 512],
                                    start=(ic == 0), stop=(ic == IC - 1),
                                )
                            nc.scalar.activation(
                                dst[:, ob, sc * 512:(sc + 1) * 512], ps[:],
                                AFT.Identity, bias=b_sb[:, ob:ob + 1], scale=1.0,
                            )

                # v: [s, 256] = x @ Wv_slice^T, via lhsT = x^T chunks
                for s_t in range(ST):
                    ps = ps_mm.tile([P, 512], F32, tag="mm", name="ps_v")
                    for ic in range(IC):
                        nc.tensor.matmul(
                            ps[:, :DG],
                            xT_sb[:, ic, s_t * P:(s_t + 1) * P],
                            wvT_sb[:, ic, :],
                            start=(ic == 0), stop=(ic == IC - 1),
                        )
                    if add_bias_v:
                        nc.vector.tensor_add(v_sb[:, s_t, :], ps[:, :DG], bv_bc[:])
                    else:
                        nc.any.tensor_copy(v_sb[:, s_t, :], ps[:, :DG])

            # phase-2/3 pools opened after wts closed so its space is reclaimed
            attn_pool = ctx.enter_context(tc.tile_pool(name="attn", bufs=2))
            stage = ctx.enter_context(tc.tile_pool(name="stage", bufs=3))
            stats = ctx.enter_context(tc.tile_pool(name="stats", bufs=4))
            outp = ctx.enter_context(tc.tile_pool(name="outp", bufs=3))

            # woT used only in phase 3
            woT_sb = outp.tile([P, IC, OC], DTO, tag="woT", bufs=1)
            nc.gpsimd.dma_start(woT_sb[:], woT.ap().rearrange("(c p) o -> p c o", p=P))

            # ---- phase 2 + 3 interleaved: q-group outer so the per-group
            # AllGather + out-projection overlap later groups' attention ----
            for g in range(NQG):
                for h in range(HG):
                    hp = 64 * (h % 2)  # partition offset of head in qT/kT blocks
                    hb = h // 2        # o-block of this head
                    attn_t = attn_pool.tile([P, 4, s], F32, tag="attn", name="attn_t")
                    for tq in range(4 * g, 4 * g + 4):
                        L = (tq + 1) * P
                        nch = (L + 511) // 512
                        row = attn_t[:, tq - 4 * g, :]
                        # online (per-chunk) softmax: each chunk exps against
                        # its own max so PSUM frees immediately; a per-chunk
                        # rescale factor fixes everything up at normalize time.
                        cmax = stats.tile([P, 4], F32, tag="cmax", name="cmax")
                        nm8c = stats.tile([P, 4], F32, tag="nm8c", name="nm8c")
                        zp = stats.tile([P, 4], F32, tag="zp", name="zp")
                        e8 = stats.tile([P, 4], F32, tag="e8", name="e8")
                        zw = stats.tile([P, 4], F32, tag="zw", name="zw")
                        fac = stats.tile([P, 4], F32, tag="fac", name="fac")
                        negmax = stats.tile([P, 1], F32, tag="negmax", name="negmax")
                        negm8 = stats.tile([P, 1], F32, tag="negm8", name="negm8")
                        z = stats.tile([P, 1], F32, tag="z", name="z")
                        invz = stats.tile([P, 1], F32, tag="invz", name="invz")
                        for kc in range(nch):
                            w = min(512, L - kc * 512)
                            ps = ps_mm.tile([P, 512], F32, tag="mm", name="ps_s")
                            nc.tensor.matmul(
                                ps[:, :w],
                                qT_sb[hp:hp + 64, hb, tq * P:(tq + 1) * P],
                                kT_sb[hp:hp + 64, hb, kc * 512:kc * 512 + w],
                                start=True, stop=True,
                            )
                            if kc == nch - 1:
                                nc.vector.tensor_add(
                                    ps[:, w - P:w], ps[:, w - P:w], cmask[:]
                                )
                            nc.vector.reduce_max(
                                cmax[:, kc:kc + 1], ps[:, :w],
                                axis=mybir.AxisListType.X,
                            )
                            nc.vector.tensor_scalar_mul(
                                nm8c[:, kc:kc + 1], cmax[:, kc:kc + 1], -SCALE
                            )
                            nc.scalar.activation(
                                row[:, kc * 512:kc * 512 + w], ps[:, :w],
                                AFT.Exp, bias=nm8c[:, kc:kc + 1], scale=SCALE,
                                accum_out=zp[:, kc:kc + 1],
                            )
                        # combine: M = max_kc cmax; e8 = exp(SCALE*(cmax-M));
                        # Z = sum e8*zp; row[kc] *= e8[kc]/Z
                        nc.vector.reduce_max(
                            negmax[:], cmax[:, :nch],
                            axis=mybir.AxisListType.X, negate=True,
                        )
                        nc.vector.tensor_scalar_mul(negm8[:], negmax[:], SCALE)
                        nc.scalar.activation(
                            e8[:, :nch], cmax[:, :nch],
                            AFT.Exp, bias=negm8[:, 0:1], scale=SCALE,
                        )
                        nc.vector.tensor_mul(zw[:, :nch], zp[:, :nch], e8[:, :nch])
                        nc.vector.reduce_sum(
                            z[:], zw[:, :nch], axis=mybir.AxisListType.X
                        )
                        nc.vector.reciprocal(invz[:], z[:])
                        nc.vector.tensor_scalar_mul(
                            fac[:, :nch], e8[:, :nch], invz[:, 0:1]
                        )
                        for kc in range(nch):
                            w = min(512, L - kc * 512)
                            nc.any.tensor_scalar_mul(
                                row[:, kc * 512:kc * 512 + w],
                                row[:, kc * 512:kc * 512 + w],
                                fac[:, kc:kc + 1],
                            )
                        nc.sync.dma_start(
                            ao[h, tq * P:(tq + 1) * P, :L], row[:, :L]
                        )

                    # AV: ctx^T[64, 512] for this q-group, contract over k-tiles
                    ctx_ps = ps_ctx.tile([64, 512], F32, tag="ctx", name="ctx_ps")
                    for j in range(4 * g + 4):
                        t0 = max(0, j - 4 * g)  # first valid local q-tile
                        stg = stage.tile([P, 512], DTO, tag="stg", name="stg")
                        tp = ps_tp.tile([P, 512], F32, tag="tp", name="tp")
                        for tl in range(t0, 4):
                            nc.tensor.transpose(
                                tp[:, tl * P:(tl + 1) * P],
                                attn_t[:, tl, j * P:(j + 1) * P],
                                identity[:],
                            )
                        if t0 > 0:
                            nc.gpsimd.memset(
                                stg[:, :t0 * P].bitcast(mybir.dt.uint32), 0
                            )
                            nc.any.tensor_copy(stg[:, t0 * P:], tp[:, t0 * P:])
                        else:
                            nc.any.tensor_copy(stg[:], tp[:])
                        nc.tensor.matmul(
                            ctx_ps[:],
                            v_sb[:, j, h * 64:(h + 1) * 64],
                            stg[:],
                            start=(j == 0), stop=(j == 4 * g + 3),
                        )
                    nc.any.tensor_copy(
                        ctxT_sb[hp:hp + 64, hb, g * 512:(g + 1) * 512], ctx_ps[:]
                    )

                # ---- per-group AllGather + out-projection (columns slice) ----
                for blk in range(2):
                    nc.sync.dma_start(
                        cc_in[g].ap()[blk * P:(blk + 1) * P, :],
                        ctxT_sb[:, blk, g * 512:(g + 1) * 512],
                    )
                nc.gpsimd.collective_compute(
                    "AllGather", mybir.AluOpType.bypass, replica_groups=RG,
                    ins=[cc_in[g].ap().opt()], outs=[cc_out[g].ap().opt()],
                )
                ccv = cc_out[g].ap().rearrange("(c p) t -> p c t", p=P)
                for st_l in range(4):
                    ctx_l = outp.tile([P, IC, P], DTO, tag="ctxl", name="ctx_l")
                    nc.gpsimd.dma_start(
                        ctx_l[:], ccv[:, :, st_l * P:(st_l + 1) * P]
                    )
                    ps = ps_mm.tile([P, 512], F32, tag="mm", name="ps_o")
                    for ic in range(IC):
                        nc.tensor.matmul(
                            ps[:, :OC],
                            ctx_l[:, ic, :],
                            woT_sb[:, ic, :],
                            start=(ic == 0), stop=(ic == IC - 1),
                        )
                    ot = outp.tile([P, OC], F32, tag="ot", name="ot")
                    if add_bias_o:
                        nc.vector.tensor_add(ot[:], ps[:, :OC], bo_bc[:])
                    else:
                        nc.any.tensor_copy(ot[:], ps[:, :OC])
                    s_t = g * 4 + st_l
                    nc.sync.dma_start(oo[s_t * P:(s_t + 1) * P, :], ot[:])

    nc.compile()
    return nc


def make_in_maps(x, Wq, bq, Wk, bk, Wv, bv, Wo, bo, s=S):
    """Host-side sharding: slicing + layout transposes only."""
    x = np.asarray(x, dtype=np.float32)
    OC = HID // CPB
    in_maps = []
    xT = [np.ascontiguousarray(x[b].T) for b in range(B)]
    for c in range(N_CORES):
        b, hg = c // CPB, c % CPB
        sl = slice(hg * DG, (hg + 1) * DG)
        osl = slice(hg * OC, (hg + 1) * OC)
        in_maps.append({
            "xT": xT[b],
            "wqT": np.ascontiguousarray(np.asarray(Wq, np.float32)[sl].T),
            "wkT": np.ascontiguousarray(np.asarray(Wk, np.float32)[sl].T),
            "wvT": np.ascontiguousarray(np.asarray(Wv, np.float32)[sl].T),
            "woT": np.ascontiguousarray(np.asarray(Wo, np.float32)[osl].T),
            "bq": np.ascontiguousarray(np.asarray(bq, np.float32)[sl]),
            "bk": np.ascontiguousarray(np.asarray(bk, np.float32)[sl]),
            "bv": np.ascontiguousarray(np.asarray(bv, np.float32)[sl]),
            "bo": np.ascontiguousarray(np.asarray(bo, np.float32)[osl]),
        })
    return in_maps


def assemble(results, s=S):
    OC = HID // CPB
    out = np.empty((B, s, HID), dtype=np.float32)
    attn = np.empty((B, NH, s, s), dtype=np.float32)
    for c in range(N_CORES):
        b, hg = c // CPB, c % CPB
        attn[b, hg * HG:(hg + 1) * HG] = results[c]["attn_o"]
        out[b, :, hg * OC:(hg + 1) * OC] = results[c]["out_o"]
    return out, attn


def kernel(x, Wq, bq, Wk, bk, Wv, bv, Wo, bo):
    global LAST_RESULT
    add_bias_v = bool(np.any(np.asarray(bv)))
    add_bias_o = bool(np.any(np.asarray(bo)))
    nc = _build(add_bias_v=add_bias_v, add_bias_o=add_bias_o)
    in_maps = make_in_maps(x, Wq, bq, Wk, bk, Wv, bv, Wo, bo)
    res = bass_utils.run_bass_kernel_spmd(
        nc, in_maps, core_ids=list(range(N_CORES)),
    )
    LAST_RESULT = res
    return assemble(res.results)


# revision 22
# speedup vs baseline: 2.7380x; 1.1808x over previous
"""Distributed Bass kernel for nn_AttentionLayer (B=2, S=2048, HID=1024, NH=16).

Sharding: core c of 8 handles batch b = c//4 and heads [4*(c%4), 4*(c%4)+4).
 - QKV projections + causal attention computed fully locally per (batch, head-group).
 - attn probabilities ([B, NH, S, S]) written directly to per-core output shards.
 - ctx^T shards exchanged with an AllToAll over 4-core batch groups, after which
   each core computes a disjoint 512-row slice of the output projection.

Host-side work is layout only: inputs are sliced/transposed per core, outputs
are concatenated. All FLOPs run on the NeuronCores.
"""

import math
import os
from contextlib import ExitStack

import numpy as np

import concourse.bass as bass
import concourse.tile as tile
from concourse import bacc, bass_utils, mybir
from concourse.masks import make_identity

# ---- problem constants (hardcoded per harness) ----
B, S, HID, NH, HD = 2, 2048, 1024, 16, 64
N_CORES, CPB = 8, 4  # total cores, cores per batch
HG = NH // CPB  # heads per core = 4
DG = HG * HD  # ctx dims per core = 256
P = 128
F32 = mybir.dt.float32
F32R = mybir.dt.float32r
NEG_BIG = -1.0e30
SCALE = float(HD) ** 0.5  # reference: scores = qk / (HD ** -0.5) == qk * 8
RG = [[0, 1, 2, 3], [4, 5, 6, 7]]
AFT = mybir.ActivationFunctionType

# runtime-config (test.py may override via env)
USE_FP32R_SCORES = os.environ.get("K_FP32R_SCORES", "1") == "1"
USE_FP32R_OTHER = os.environ.get("K_FP32R_OTHER", "1") == "1"

LAST_RESULT = None  # test.py reads exec_time_ns from here


def _build(s=S, fp32r_scores=USE_FP32R_SCORES, fp32r_other=USE_FP32R_OTHER,
           add_bias_v=False, add_bias_o=False):
    ST = s // P          # s-tiles
    SC = s // 512        # 512-wide chunks
    IC = HID // P        # hid chunks = 8
    NQG = ST // 4        # q groups (4 tiles each)
    RS = s // CPB        # out rows per core
    RT = RS // P

    DTS = F32R if fp32r_scores else F32   # scores matmul operand dtype
    DTO = F32R if fp32r_other else F32    # other matmul operand dtype

    nc = bacc.Bacc("TRN2", debug=False, num_devices=N_CORES)

    xT = nc.dram_tensor("xT", [HID, s], F32, kind="ExternalInput")
    wqT = nc.dram_tensor("wqT", [HID, DG], F32, kind="ExternalInput")
    wkT = nc.dram_tensor("wkT", [HID, DG], F32, kind="ExternalInput")
    wvT = nc.dram_tensor("wvT", [HID, DG], F32, kind="ExternalInput")
    OC = HID // CPB  # out-projection columns per core = 256
    woT = nc.dram_tensor("woT", [HID, OC], F32, kind="ExternalInput")
    bq = nc.dram_tensor("bq", [DG], F32, kind="ExternalInput")
    bk = nc.dram_tensor("bk", [DG], F32, kind="ExternalInput")
    bv = nc.dram_tensor("bv", [DG], F32, kind="ExternalInput")
    bo = nc.dram_tensor("bo", [OC], F32, kind="ExternalInput")
    attn_o = nc.dram_tensor("attn_o", [HG, s, s], F32, kind="ExternalOutput")
    out_o = nc.dram_tensor("out_o", [s, OC], F32, kind="ExternalOutput")
    # per-q-group collective bounce buffers (contiguous per chunk)
    cc_in = [nc.dram_tensor(f"cc_in{g}", [DG, 4 * P], F32) for g in range(s // P // 4)]
    cc_out = [
        nc.dram_tensor(f"cc_out{g}", [CPB * DG, 4 * P], F32) for g in range(s // P // 4)
    ]

    ao = attn_o.ap()
    oo = out_o.ap()

    with tile.TileContext(nc) as tc:
        with ExitStack() as ctx:
            consts = ctx.enter_context(tc.tile_pool(name="consts", bufs=1))
            acts = ctx.enter_context(tc.tile_pool(name="acts", bufs=1))
            ps_mm = ctx.enter_context(tc.tile_pool(name="ps_mm", bufs=4, space="PSUM"))
            ps_tp = ctx.enter_context(tc.tile_pool(name="ps_tp", bufs=2, space="PSUM"))
            ps_ctx = ctx.enter_context(tc.tile_pool(name="ps_ctx", bufs=2, space="PSUM"))

            # ---- constants ----
            identity = consts.tile([P, P], F32)
            make_identity(nc, identity[:])
            cmask = consts.tile([P, P], F32)
            nc.gpsimd.memset(cmask[:], 0.0)
            # cmask[x, y] = (x - y >= 0) ? 0 : NEG_BIG   (mask strictly-upper)
            nc.gpsimd.affine_select(
                out=cmask[:], in_=cmask[:],
                compare_op=mybir.AluOpType.is_ge, fill=NEG_BIG,
                base=0, pattern=[[-1, P]], channel_multiplier=1,
            )
            bq_sb = consts.tile([P, 2], F32)
            nc.sync.dma_start(bq_sb[:], bq.ap().rearrange("(o p) -> p o", p=P))
            bk_sb = consts.tile([P, 2], F32)
            nc.sync.dma_start(bk_sb[:], bk.ap().rearrange("(o p) -> p o", p=P))
            bv_bc = bo_bc = None
            if add_bias_v:
                bv_bc = consts.tile([P, DG], F32)
                nc.sync.dma_start(bv_bc[0:1, :], bv.ap()[None, :])
                nc.gpsimd.partition_broadcast(bv_bc[:], bv_bc[0:1, :])
            if add_bias_o:
                bo_bc = consts.tile([P, OC], F32)
                nc.sync.dma_start(bo_bc[0:1, :], bo.ap()[None, :])
                nc.gpsimd.partition_broadcast(bo_bc[:], bo_bc[0:1, :])

            # ---- persistent activations ----
            qT_sb = acts.tile([P, 2, s], DTS)  # q^T  [head-dim part, s]
            kT_sb = acts.tile([P, 2, s], DTS)
            v_sb = acts.tile([P, ST, DG], DTO)  # v    [s part, head dims]
            ctxT_sb = acts.tile([P, 2, s], F32)  # ctx^T

            # ---- phase 1: load xT + weights, projections ----
            with tc.tile_pool(name="wts", bufs=1) as wts:
                xT_sb = wts.tile([P, IC, s], DTO)
                nc.gpsimd.dma_start(xT_sb[:], xT.ap().rearrange("(c p) t -> p c t", p=P))
                wqT_sb = wts.tile([P, IC, DG], DTO)
                nc.gpsimd.dma_start(wqT_sb[:], wqT.ap().rearrange("(c p) o -> p c o", p=P))
                wkT_sb = wts.tile([P, IC, DG], DTO)
                nc.gpsimd.dma_start(wkT_sb[:], wkT.ap().rearrange("(c p) o -> p c o", p=P))
                wvT_sb = wts.tile([P, IC, DG], DTO)
                nc.gpsimd.dma_start(wvT_sb[:], wvT.ap().rearrange("(c p) o -> p c o", p=P))

                # q^T, k^T: [256, s] = W_slice @ x^T, via lhsT = W^T chunks
                for wT_sb, dst, b_sb in ((wqT_sb, qT_sb, bq_sb), (wkT_sb, kT_sb, bk_sb)):
                    for ob in range(2):
                        for sc in range(SC):
                            ps = ps_mm.tile([P, 512], F32, tag="mm", name="ps_proj")
                            for ic in range(IC):
                                nc.tensor.matmul(
                                    ps[:],
                                    wT_sb[:, ic, ob * P:(ob + 1) * P],
                                    xT_sb[:, ic, sc * 512:(sc + 1) * 512],
                                    start=(ic == 0), stop=(ic == IC - 1),
                                )
                            nc.scalar.activation(
                                dst[:, ob, sc * 512:(sc + 1) * 512], ps[:],
                                AFT.Identity, bias=b_sb[:, ob:ob + 1], scale=1.0,
                            )

                # v: [s, 256] = x @ Wv_slice^T, via lhsT = x^T chunks
                for s_t in range(ST):
                    ps = ps_mm.tile([P, 512], F32, tag="mm", name="ps_v")
                    for ic in range(IC):
                        nc.tensor.matmul(
                            ps[:, :DG],
                            xT_sb[:, ic, s_t * P:(s_t + 1) * P],
                            wvT_sb[:, ic, :],
                            start=(ic == 0), stop=(ic == IC - 1),
                        )
                    if add_bias_v:
                        nc.vector.tensor_add(v_sb[:, s_t, :], ps[:, :DG], bv_bc[:])
                    else:
                        nc.any.tensor_copy(v_sb[:, s_t, :], ps[:, :DG])

            # phase-2/3 pools opened after wts closed so its space is reclaimed
            attn_pool = ctx.enter_context(tc.tile_pool(name="attn", bufs=2))
            stage = ctx.enter_context(tc.tile_pool(name="stage", bufs=3))
            stats = ctx.enter_context(tc.tile_pool(name="stats", bufs=4))
            outp = ctx.enter_context(tc.tile_pool(name="outp", bufs=3))

            # woT used only in phase 3
            woT_sb = outp.tile([P, IC, OC], DTO, tag="woT", bufs=1)
            nc.gpsimd.dma_start(woT_sb[:], woT.ap().rearrange("(c p) o -> p c o", p=P))

            def emit_outproj(g):
                ccv = cc_out[g].ap().rearrange("(c p) t -> p c t", p=P)
                for st_l in range(4):
                    ctx_l = outp.tile([P, IC, P], DTO, tag="ctxl", name="ctx_l")
                    nc.gpsimd.dma_start(
                        ctx_l[:], ccv[:, :, st_l * P:(st_l + 1) * P]
                    )
                    ps = ps_mm.tile([P, 512], F32, tag="mm", name="ps_o")
                    for ic in range(IC):
                        nc.tensor.matmul(
                            ps[:, :OC],
                            ctx_l[:, ic, :],
                            woT_sb[:, ic, :],
                            start=(ic == 0), stop=(ic == IC - 1),
                        )
                    ot = outp.tile([P, OC], F32, tag="ot", name="ot")
                    if add_bias_o:
                        nc.vector.tensor_add(ot[:], ps[:, :OC], bo_bc[:])
                    else:
                        nc.any.tensor_copy(ot[:], ps[:, :OC])
                    s_t = g * 4 + st_l
                    nc.sync.dma_start(oo[s_t * P:(s_t + 1) * P, :], ot[:])

            # ---- phase 2 + 3 interleaved: q-group outer so the per-group
            # AllGather + out-projection overlap later groups' attention ----
            for g in range(NQG):
                for h in range(HG):
                    hp = 64 * (h % 2)  # partition offset of head in qT/kT blocks
                    hb = h // 2        # o-block of this head
                    attn_t = attn_pool.tile([P, 4, s], F32, tag="attn", name="attn_t")
                    for tq in range(4 * g, 4 * g + 4):
                        L = (tq + 1) * P
                        nch = (L + 511) // 512
                        row = attn_t[:, tq - 4 * g, :]
                        # online (per-chunk) softmax: each chunk exps against
                        # its own max so PSUM frees immediately; a per-chunk
                        # rescale factor fixes everything up at normalize time.
                        cmax = stats.tile([P, 4], F32, tag="cmax", name="cmax")
                        nm8c = stats.tile([P, 4], F32, tag="nm8c", name="nm8c")
                        zp = stats.tile([P, 4], F32, tag="zp", name="zp")
                        e8 = stats.tile([P, 4], F32, tag="e8", name="e8")
                        zw = stats.tile([P, 4], F32, tag="zw", name="zw")
                        fac = stats.tile([P, 4], F32, tag="fac", name="fac")
                        negmax = stats.tile([P, 1], F32, tag="negmax", name="negmax")
                        negm8 = stats.tile([P, 1], F32, tag="negm8", name="negm8")
                        z = stats.tile([P, 1], F32, tag="z", name="z")
                        invz = stats.tile([P, 1], F32, tag="invz", name="invz")
                        for kc in range(nch):
                            w = min(512, L - kc * 512)
                            ps = ps_mm.tile([P, 512], F32, tag="mm", name="ps_s")
                            nc.tensor.matmul(
                                ps[:, :w],
                                qT_sb[hp:hp + 64, hb, tq * P:(tq + 1) * P],
                                kT_sb[hp:hp + 64, hb, kc * 512:kc * 512 + w],
                                start=True, stop=True,
                            )
                            if kc == nch - 1:
                                nc.vector.tensor_add(
                                    ps[:, w - P:w], ps[:, w - P:w], cmask[:]
                                )
                            nc.vector.reduce_max(
                                cmax[:, kc:kc + 1], ps[:, :w],
                                axis=mybir.AxisListType.X,
                            )
                            nc.vector.tensor_scalar_mul(
                                nm8c[:, kc:kc + 1], cmax[:, kc:kc + 1], -SCALE
                            )
                            nc.scalar.activation(
                                row[:, kc * 512:kc * 512 + w], ps[:, :w],
                                AFT.Exp, bias=nm8c[:, kc:kc + 1], scale=SCALE,
                                accum_out=zp[:, kc:kc + 1],
                            )
                        # combine: M = max_kc cmax; e8 = exp(SCALE*(cmax-M));
                        # Z = sum e8*zp; row[kc] *= e8[kc]/Z
                        nc.vector.reduce_max(
                            negmax[:], cmax[:, :nch],
                            axis=mybir.AxisListType.X, negate=True,
                        )
                        nc.vector.tensor_scalar_mul(negm8[:], negmax[:], SCALE)
                        nc.scalar.activation(
                            e8[:, :nch], cmax[:, :nch],
                            AFT.Exp, bias=negm8[:, 0:1], scale=SCALE,
                        )
                        nc.vector.tensor_mul(zw[:, :nch], zp[:, :nch], e8[:, :nch])
                        nc.vector.reduce_sum(
                            z[:], zw[:, :nch], axis=mybir.AxisListType.X
                        )
                        nc.vector.reciprocal(invz[:], z[:])
                        nc.vector.tensor_scalar_mul(
                            fac[:, :nch], e8[:, :nch], invz[:, 0:1]
                        )
                        for kc in range(nch):
                            w = min(512, L - kc * 512)
                            nc.any.tensor_scalar_mul(
                                row[:, kc * 512:kc * 512 + w],
                                row[:, kc * 512:kc * 512 + w],
                                fac[:, kc:kc + 1],
                            )
                        nc.sync.dma_start(
                            ao[h, tq * P:(tq + 1) * P, :L], row[:, :L]
                        )

                    # AV: ctx^T[64, 512] for this q-group, contract over k-tiles
                    ctx_ps = ps_ctx.tile([64, 512], F32, tag="ctx", name="ctx_ps")
                    for j in range(4 * g + 4):
                        t0 = max(0, j - 4 * g)  # first valid local q-tile
                        stg = stage.tile([P, 512], DTO, tag="stg", name="stg")
                        tp = ps_tp.tile([P, 512], F32, tag="tp", name="tp")
                        for tl in range(t0, 4):
                            nc.tensor.transpose(
                                tp[:, tl * P:(tl + 1) * P],
                                attn_t[:, tl, j * P:(j + 1) * P],
                                identity[:],
                            )
                        if t0 > 0:
                            nc.gpsimd.memset(
                                stg[:, :t0 * P].bitcast(mybir.dt.uint32), 0
                            )
                            nc.any.tensor_copy(stg[:, t0 * P:], tp[:, t0 * P:])
                        else:
                            nc.any.tensor_copy(stg[:], tp[:])
                        nc.tensor.matmul(
                            ctx_ps[:],
                            v_sb[:, j, h * 64:(h + 1) * 64],
                            stg[:],
                            start=(j == 0), stop=(j == 4 * g + 3),
                        )
                    nc.any.tensor_copy(
                        ctxT_sb[hp:hp + 64, hb, g * 512:(g + 1) * 512], ctx_ps[:]
                    )

                # ---- per-group AllGather issued right after group g's ctx ----
                for blk in range(2):
                    nc.sync.dma_start(
                        cc_in[g].ap()[blk * P:(blk + 1) * P, :],
                        ctxT_sb[:, blk, g * 512:(g + 1) * 512],
                    )
                nc.gpsimd.collective_compute(
                    "AllGather", mybir.AluOpType.bypass, replica_groups=RG,
                    ins=[cc_in[g].ap().opt()], outs=[cc_out[g].ap().opt()],
                )
                # out-projection for the PREVIOUS group — its AllGather has had
                # a full group of attention to complete, so no engine stalls on
                # the collective in the static schedule.
                if g >= 1:
                    emit_outproj(g - 1)
            emit_outproj(NQG - 1)

    nc.compile()
    return nc


def make_in_maps(x, Wq, bq, Wk, bk, Wv, bv, Wo, bo, s=S):
    """Host-side sharding: slicing + layout transposes only."""
    x = np.asarray(x, dtype=np.float32)
    OC = HID // CPB
    in_maps = []
    xT = [np.ascontiguousarray(x[b].T) for b in range(B)]
    for c in range(N_CORES):
        b, hg = c // CPB, c % CPB
        sl = slice(hg * DG, (hg + 1) * DG)
        osl = slice(hg * OC, (hg + 1) * OC)
        in_maps.append({
            "xT": xT[b],
            "wqT": np.ascontiguousarray(np.asarray(Wq, np.float32)[sl].T),
            "wkT": np.ascontiguousarray(np.asarray(Wk, np.float32)[sl].T),
            "wvT": np.ascontiguousarray(np.asarray(Wv, np.float32)[sl].T),
            "woT": np.ascontiguousarray(np.asarray(Wo, np.float32)[osl].T),
            "bq": np.ascontiguousarray(np.asarray(bq, np.float32)[sl]),
            "bk": np.ascontiguousarray(np.asarray(bk, np.float32)[sl]),
            "bv": np.ascontiguousarray(np.asarray(bv, np.float32)[sl]),
            "bo": np.ascontiguousarray(np.asarray(bo, np.float32)[osl]),
        })
    return in_maps


def assemble(results, s=S):
    OC = HID // CPB
    out = np.empty((B, s, HID), dtype=np.float32)
    attn = np.empty((B, NH, s, s), dtype=np.float32)
    for c in range(N_CORES):
        b, hg = c // CPB, c % CPB
        attn[b, hg * HG:(hg + 1) * HG] = results[c]["attn_o"]
        out[b, :, hg * OC:(hg + 1) * OC] = results[c]["out_o"]
    return out, attn


def kernel(x, Wq, bq, Wk, bk, Wv, bv, Wo, bo):
    global LAST_RESULT
    add_bias_v = bool(np.any(np.asarray(bv)))
    add_bias_o = bool(np.any(np.asarray(bo)))
    nc = _build(add_bias_v=add_bias_v, add_bias_o=add_bias_o)
    in_maps = make_in_maps(x, Wq, bq, Wk, bk, Wv, bv, Wo, bo)
    res = bass_utils.run_bass_kernel_spmd(
        nc, in_maps, core_ids=list(range(N_CORES)),
    )
    LAST_RESULT = res
    return assemble(res.results)
